# revision 6
# baseline (speedup 1.0000x reference)
"""CharCNN embedder (ELMo-style) Trainium2 Bass kernel, v2.

Strategy (pure data parallel over 8 cores, 256 tokens each):
  - Embedding lookup as one-hot matmul (fp16): ids replicated across
    partitions via a K=1 ones-matmul, one-hot via is_equal (DVE), then
    embT.T @ onehot over 3 row-chunks -> xT [16, 12800] in conv layout.
  - im2col by 6 shifted SBUF->SBUF DMA copies -> X7 [118, 12800+pad] with
    6 indicator rows carrying -1e30 in the conv weights for invalid
    (channel, position) pairs so max-over-time needs no masking.
  - All 7 convs as one packed [118, 2048] bf16 matmul per (group, m-chunk).
    Max-over-time drained by THREE engines in parallel:
      * 7 m-chunks: DVE reduce_max straight from PSUM.
      * 9 m-chunks: Scalar ACT(Relu+bias) PSUM->SBUF bf16, two GpSimd
        overlapping-window TT-max tree levels, small DVE final reduce.
    (relu commutes with max; bias is constant over positions.)
  - Highway weight-stationary and channel-major: out[128 ch, 256 tok]
    accumulated over 16 K-chunks; bias fused into Scalar Relu/Sigmoid;
    gate math as three bf16 DVE tensor_tensor ops. No transposes at all.
  - Projection token-major (activations stationary) as before.
"""

import os
import numpy as np
import ml_dtypes

import concourse.bass as bass
import concourse.mybir as mybir
import concourse.tile as tile
from concourse.bass_utils import run_bass_kernel_spmd

F32 = mybir.dt.float32
BF16 = mybir.dt.bfloat16
FP16 = mybir.dt.float16
NPBF16 = ml_dtypes.bfloat16

CNN_OPTIONS = [(1, 32), (2, 32), (3, 64), (4, 128), (5, 256), (6, 512), (7, 1024)]
EMB_DIM = 16
N_CHARS = 262
MAX_CHARS = 50
N_FILTERS = 2048
OUT_DIM = 512
N_HIGHWAY = 2
BATCH, SEQ = 4, 512
NCORES = 8
T_LOC = BATCH * SEQ // NCORES          # 256 tokens per core
COLS = T_LOC * MAX_CHARS               # 12800
COLS_PAD = COLS + 16                   # 12816
KMAX = 7
KROWS = EMB_DIM * KMAX                 # 112
KTOT = KROWS + 6                       # 118 (6 indicator rows for pos 44..49)
NCH = 512                              # xT build chunk width
NXCH = COLS // NCH                     # 25
TOKG = 8                               # tokens per conv chunk
NTG = T_LOC // TOKG                    # 32 conv N-chunks
# oc-chunk list: chunk idx -> kernel size driving its valid-position count
CHUNK_K = [1, 4, 5, 5, 6, 6, 6, 6, 7, 7, 7, 7, 7, 7, 7, 7]
CHUNK_NP = [50 if k == 1 else (MAX_CHARS - k + 1) for k in CHUNK_K]
# drain class: True -> DVE reduce_max direct from PSUM; False -> S+V staged path
DIRECT_V = [True] * 16
KC = 16                                # 2048/128 contraction chunks
NQ = 32                                # highway out-chunks (4096/128)


def _half(n):
    return (n + 1) // 2


def _split_multi_waits(nc):
    """This walrus build encodes at most ONE sync-wait per instruction.
    Hoist extra waits onto dedicated NoOps ahead of the instruction."""
    ctr = [0]
    for f in nc.m.functions:
        for b in f.blocks:
            il = b.instructions
            if not any(
                i.sync_info is not None and len(i.sync_info.on_wait) > 1 for i in il
            ):
                continue
            new = []
            for ins in il:
                si = ins.sync_info
                if si is not None and len(si.on_wait) > 1:
                    waits = list(si.on_wait)
                    for w in waits[:-1]:
                        ctr[0] += 1
                        nop = mybir.InstNoOp(name=f"wsplit-{ctr[0]}", ins=[], outs=[])
                        nop.engine = ins.engine
                        nop.sync_info = mybir.SyncInfo(on_wait=[w], on_update=[])
                        new.append(nop)
                    ins.sync_info = mybir.SyncInfo(
                        on_wait=[waits[-1]], on_update=list(si.on_update)
                    )
                new.append(ins)
            b.instructions = new


def _build_program():
    nc = bass.Bass(target_bir_lowering=False)

    ids_d = nc.dram_tensor("ids", [1, COLS_PAD], FP16, kind="ExternalInput")
    iota_d = nc.dram_tensor("iota3", [128, 4], F32, kind="ExternalInput")
    embt_d = nc.dram_tensor("embt", [384, EMB_DIM], FP16, kind="ExternalInput")
    convw_d = nc.dram_tensor("convw", [KTOT, N_FILTERS], BF16, kind="ExternalInput")
    indic_d = nc.dram_tensor("indic", [6, COLS_PAD], BF16, kind="ExternalInput")
    cbias_d = nc.dram_tensor("cbias", [128, 16], F32, kind="ExternalInput")
    hw0_d = nc.dram_tensor("hw0", [NQ, 128, KC * 128], BF16, kind="ExternalInput")
    hw1_d = nc.dram_tensor("hw1", [NQ, 128, KC * 128], BF16, kind="ExternalInput")
    hb0_d = nc.dram_tensor("hb0", [128, NQ], F32, kind="ExternalInput")
    hb1_d = nc.dram_tensor("hb1", [128, NQ], F32, kind="ExternalInput")
    pw_d = nc.dram_tensor("pw", [KC, 128, 512], BF16, kind="ExternalInput")
    pb_d = nc.dram_tensor("pb", [1, 512], BF16, kind="ExternalInput")
    out_d = nc.dram_tensor("outT", [T_LOC, OUT_DIM], F32, kind="ExternalOutput")

    with tile.TileContext(nc) as tc:
        with (
            tc.tile_pool(name="const", bufs=1) as cpool,
            tc.tile_pool(name="oh", bufs=3) as ohpool,
            tc.tile_pool(name="idsrep", bufs=3) as idpool,
            tc.tile_pool(name="stg", bufs=6) as stgpool,
            tc.tile_pool(name="tre", bufs=6) as trepool,
            tc.tile_pool(name="wslab", bufs=8) as wpool,
            tc.tile_pool(name="elem", bufs=4) as epool,
            tc.tile_pool(name="outp", bufs=2) as outpool,
            tc.tile_pool(name="ps_big", bufs=3, space="PSUM") as ps_big,
            tc.tile_pool(name="ps_xt", bufs=2, space="PSUM") as ps_xt,
            tc.tile_pool(name="ps_hw", bufs=3, space="PSUM") as ps_hw,
        ):
            # ---- constants in ----
            ids_s = cpool.tile([1, COLS_PAD], FP16, tag="ids")
            nc.sync.dma_start(ids_s[:], ids_d[:])
            iota_s = cpool.tile([128, 4], F32, tag="iota")
            nc.sync.dma_start(iota_s[:], iota_d[:])
            embt_s = cpool.tile([128, 3 * EMB_DIM], FP16, tag="embt")
            for r in range(3):
                nc.sync.dma_start(
                    embt_s[:, 16 * r : 16 * r + 16], embt_d[128 * r : 128 * r + 128, :]
                )
            convw_s = cpool.tile([KTOT, N_FILTERS], BF16, tag="convw")
            nc.sync.dma_start(convw_s[:], convw_d[:])
            cbias_s = cpool.tile([128, 16], F32, tag="cbias")
            nc.sync.dma_start(cbias_s[:], cbias_d[:])
            hb0_s = cpool.tile([128, NQ], F32, tag="hb0")
            nc.sync.dma_start(hb0_s[:], hb0_d[:])
            hb1_s = cpool.tile([128, NQ], F32, tag="hb1")
            nc.sync.dma_start(hb1_s[:], hb1_d[:])
            pb_s = cpool.tile([1, 512], BF16, tag="pb")
            nc.sync.dma_start(pb_s[:], pb_d[:])
            ones_s = cpool.tile([1, 128], FP16, tag="ones")
            nc.gpsimd.memset(ones_s[:], 1.0)
            onesb_s = cpool.tile([1, 128], BF16, tag="onesb")
            nc.gpsimd.memset(onesb_s[:], 1.0)

            # X7: rows 0-15 xT base, 16-111 shifted copies, 112-117 indicators
            X7 = cpool.tile([KTOT, COLS_PAD], BF16, tag="X7")
            nc.sync.dma_start(X7[112:118, :], indic_d[:])
            nc.gpsimd.memset(X7[0:16, COLS:COLS_PAD], 0.0)

            # channel-major activations [128 ch, 256 tok]
            hts = [cpool.tile([128, T_LOC], BF16, tag=f"hT{m}", name=f"hT{m}") for m in range(KC)]
            h1ts = [cpool.tile([128, T_LOC], BF16, tag=f"h1T{m}", name=f"h1T{m}") for m in range(KC)]
            h2ts = [cpool.tile([128, T_LOC], BF16, tag=f"h2T{m}", name=f"h2T{m}") for m in range(KC)]

            # ---- embedding xT build ----
            def bphase(lo, hi):
                for n in range(lo, hi):
                    c0 = n * NCH
                    ps_ids = ps_xt.tile(
                        [128, NCH], F32, space="PSUM", tag="xt", name=f"pid{n}"
                    )
                    nc.tensor.matmul(
                        ps_ids[:], ones_s[0:1, :], ids_s[0:1, c0 : c0 + NCH],
                        start=True, stop=True,
                    )
                    idr = idpool.tile([128, NCH], FP16, tag="idr", name=f"idr{n}")
                    nc.scalar.copy(idr[:], ps_ids[:])
                    ohs = []
                    for r in range(3):
                        oh = ohpool.tile([128, NCH], FP16, tag=f"oh{r}", name=f"oh{r}_{n}")
                        nc.vector.tensor_scalar(
                            out=oh[:], in0=idr[:], scalar1=iota_s[:, r : r + 1],
                            scalar2=None, op0=mybir.AluOpType.is_equal,
                        )
                        ohs.append(oh)
                    px = ps_xt.tile([16, NCH], F32, space="PSUM", tag="xt", name=f"px{n}")
                    for r in range(3):
                        nc.tensor.matmul(
                            px[:], embt_s[:, 16 * r : 16 * r + 16], ohs[r][:],
                            start=(r == 0), stop=(r == 2),
                        )
                    nc.scalar.copy(X7[0:16, c0 : c0 + NCH], px[:])

            def shifts(cl, cu):
                for j in range(1, KMAX):
                    nc.sync.dma_start(
                        X7[16 * j : 16 * j + 16, cl:cu], X7[0:16, cl + j : cu + j]
                    )

            # ---- conv + 3-engine max-over-time drain ----
            def conv_range(nlo, nhi):
                for nn in range(nlo, nhi):
                    c0 = nn * TOKG * MAX_CHARS
                    for m in range(16):
                        npos = CHUNK_NP[m]
                        fd = TOKG * npos
                        ps = ps_big.tile(
                            [128, fd], F32, space="PSUM", tag="big",
                            name=f"cv{m}_{nn}",
                        )
                        rhs = (
                            X7[0:KTOT, c0 : c0 + TOKG * MAX_CHARS]
                            .rearrange("p (t c) -> p t c", c=MAX_CHARS)[:, :, 0:npos]
                        )
                        nc.tensor.matmul(
                            ps[:], convw_s[:, 128 * m : 128 * m + 128], rhs,
                            start=True, stop=True,
                        )
                        dst = hts[m][:, TOKG * nn : TOKG * nn + TOKG]
                        if DIRECT_V[m]:
                            nc.vector.reduce_max(
                                dst,
                                ps[:].rearrange("p (t c) -> p t c", c=npos),
                                axis=mybir.AxisListType.X,
                            )
                        else:
                            stg = stgpool.tile(
                                [128, fd], BF16, tag="stg", name=f"st{m}_{nn}"
                            )
                            nc.scalar.activation(
                                stg[:], ps[:], mybir.ActivationFunctionType.Relu,
                                bias=cbias_s[:, m : m + 1], scale=1.0,
                            )
                            nc.vector.reduce_max(
                                dst,
                                stg[:].rearrange("p (t c) -> p t c", c=npos),
                                axis=mybir.AxisListType.X,
                            )

            def finish_direct():
                # bias+relu for the direct-V chunks (S+G+V path already did it)
                for m in range(16):
                    if DIRECT_V[m]:
                        nc.scalar.activation(
                            hts[m][:], hts[m][:], mybir.ActivationFunctionType.Relu,
                            bias=cbias_s[:, m : m + 1], scale=1.0,
                        )

            # ---- highway, weight-stationary / channel-major ----
            def hw_layer(lyr):
                src_ts = hts if lyr == 0 else h1ts
                dst_ts = h1ts if lyr == 0 else h2ts
                w_d = hw0_d if lyr == 0 else hw1_d
                hb_s = hb0_s if lyr == 0 else hb1_s
                pair_ps = [None, None]
                for q in range(NQ):
                    slab = wpool.tile(
                        [128, KC * 128], BF16, tag="wslab", name=f"ws{lyr}_{q}"
                    )
                    nc.sync.dma_start(slab[:], w_d[q])
                    ps = ps_hw.tile(
                        [128, T_LOC], F32, space="PSUM", tag="hwp",
                        name=f"hwp{lyr}_{q}",
                    )
                    for k in range(KC):
                        nc.tensor.matmul(
                            ps[:], slab[:, 128 * k : 128 * k + 128],
                            src_ts[k][:, 0:T_LOC],
                            start=(k == 0), stop=(k == KC - 1),
                        )
                    pair_ps[q % 2] = ps
                    if q % 2 == 1:
                        c = q // 2
                        r_ = epool.tile([128, T_LOC], BF16, tag="relu", name=f"r{lyr}_{c}")
                        nc.scalar.activation(
                            r_[:], pair_ps[0][:], mybir.ActivationFunctionType.Relu,
                            bias=hb_s[:, 2 * c : 2 * c + 1], scale=1.0,
                        )
                        g_ = epool.tile([128, T_LOC], BF16, tag="gate", name=f"g{lyr}_{c}")
                        nc.scalar.activation(
                            g_[:], pair_ps[1][:], mybir.ActivationFunctionType.Sigmoid,
                            bias=hb_s[:, 2 * c + 1 : 2 * c + 2], scale=1.0,
                        )
                        t1 = epool.tile([128, T_LOC], BF16, tag="t1", name=f"c1_{lyr}_{c}")
                        nc.vector.tensor_tensor(
                            out=t1[:], in0=src_ts[c][:, 0:T_LOC], in1=r_[:],
                            op=mybir.AluOpType.subtract,
                        )
                        t2 = epool.tile([128, T_LOC], BF16, tag="t2", name=f"c2_{lyr}_{c}")
                        nc.vector.tensor_tensor(
                            out=t2[:], in0=g_[:], in1=t1[:], op=mybir.AluOpType.mult
                        )
                        nc.vector.tensor_tensor(
                            out=dst_ts[c][:, 0:T_LOC], in0=t2[:], in1=r_[:],
                            op=mybir.AluOpType.add,
                        )

            # ---- schedule ----
            bphase(0, 13)
            shifts(0, 6400)
            bphase(13, NXCH)
            shifts(6400, COLS)
            conv_range(0, NTG)
            finish_direct()
            hw_layer(0)
            hw_layer(1)

            # ---- projection (token-major out) ----
            pslabs = []
            for k in range(KC):
                slab = wpool.tile([128, 512], BF16, tag="wslab", name=f"pws{k}")
                nc.sync.dma_start(slab[:], pw_d[k])
                pslabs.append(slab)
            for mt in range(2):
                ps = ps_hw.tile([128, 512], F32, space="PSUM", tag="hwp", name=f"pj{mt}")
                for k in range(KC):
                    nc.tensor.matmul(
                        ps[:], h2ts[k][:, 128 * mt : 128 * mt + 128], pslabs[k][:],
                        start=(k == 0), stop=False,
                    )
                nc.tensor.matmul(
                    ps[:], onesb_s[0:1, :], pb_s[0:1, :], start=False, stop=True
                )
                oc = outpool.tile([128, 512], F32, tag="out", name=f"oc{mt}")
                nc.scalar.copy(oc[:], ps[:])
                nc.sync.dma_start(out_d[128 * mt : 128 * mt + 128, :], oc[:])

    _split_multi_waits(nc)
    return nc


def _prep_weights(inputs):
    conv_ws = [np.asarray(inputs[f"conv_w{i}"], np.float32) for i in range(7)]
    conv_bs = [np.asarray(inputs[f"conv_b{i}"], np.float32) for i in range(7)]

    W7 = np.zeros((KTOT, N_FILTERS), np.float32)
    o0 = 0
    for (ksz, oc), w in zip(CNN_OPTIONS, conv_ws):
        for j in range(ksz):
            W7[16 * j : 16 * j + 16, o0 : o0 + oc] = w[:, :, j].T
        # indicator-mask rows: position p = 44+i invalid iff p > 50-ksz
        for i in range(6):
            if (44 + i) > (MAX_CHARS - ksz):
                W7[KROWS + i, o0 : o0 + oc] = -1e30
        o0 += oc

    b_all = np.concatenate(conv_bs)
    cbias = b_all.reshape(16, 128).T.astype(np.float32)

    indic = np.zeros((6, COLS_PAD), np.float32)
    for i in range(6):
        indic[i, (44 + i) : COLS : MAX_CHARS] = 1.0

    emb = np.asarray(inputs["emb"], np.float32)
    embt = np.zeros((384, EMB_DIM), np.float32)
    embt[:N_CHARS] = emb

    iota = np.zeros((128, 4), np.float32)
    for r in range(3):
        iota[:, r] = np.arange(128) + 128 * r
    iota[:, 3] = 1000.0  # never matches

    # weight-stationary highway slabs: out-chunk q=2c -> nonlin rows of block c,
    # q=2c+1 -> gate rows; slab[q][i, 128k+o] = W[rows_q[o], 128k+i]
    def hw_slabs(w, bvec):
        W = np.asarray(w, np.float32)           # [4096, 2048]
        bv = np.asarray(bvec, np.float32)       # [4096]
        slabs = np.zeros((NQ, 128, KC * 128), np.float32)
        hb = np.zeros((128, NQ), np.float32)
        for q in range(NQ):
            c = q // 2
            base = 128 * c if q % 2 == 0 else N_FILTERS + 128 * c
            rows = np.arange(base, base + 128)
            Wq = W[rows]                        # [128 out, 2048 in]
            slabs[q] = Wq.T.reshape(KC, 128, 128).transpose(1, 0, 2).reshape(128, KC * 128)
            hb[:, q] = bv[rows]
        return slabs.astype(NPBF16), hb

    hw0, hb0 = hw_slabs(inputs["hw_w0"], inputs["hw_b0"])
    hw1, hb1 = hw_slabs(inputs["hw_w1"], inputs["hw_b1"])
    pwt = np.asarray(inputs["proj_w"], np.float32).T  # [2048, 512]
    pw = np.ascontiguousarray(pwt.reshape(KC, 128, 512)).astype(NPBF16)
    pb = np.asarray(inputs["proj_b"], np.float32)[None, :].astype(NPBF16)

    return {
        "iota3": iota,
        "embt": embt.astype(np.float16),
        "convw": W7.astype(NPBF16),
        "indic": indic.astype(NPBF16),
        "cbias": cbias,
        "hw0": hw0,
        "hw1": hw1,
        "hb0": hb0,
        "hb1": hb1,
        "pw": pw,
        "pb": pb,
    }


_NC_CACHE = []
LAST_RESULT = {}


def kernel(**inputs) -> np.ndarray:
    if not _NC_CACHE:
        _NC_CACHE.append(_build_program())
    nc = _NC_CACHE[0]

    shared = _prep_weights(inputs)
    ids = np.asarray(inputs["batch_ids"]).astype(np.int64).reshape(-1, MAX_CHARS)
    in_maps = []
    for core in range(NCORES):
        flat = ids[core * T_LOC : (core + 1) * T_LOC].reshape(-1)
        idsp = np.zeros((1, COLS_PAD), np.float16)
        idsp[0, :COLS] = flat.astype(np.float16)
        in_maps.append({"ids": idsp, **shared})

    trace = bool(int(os.environ.get("KERNEL_TRACE", "0")))
    res = run_bass_kernel_spmd(
        nc, in_maps, core_ids=list(range(NCORES)), trace=trace
    )
    LAST_RESULT["exec_time_ns"] = res.exec_time_ns
    LAST_RESULT["trace"] = res.instructions_and_trace

    parts = [res.results[c]["outT"] for c in range(NCORES)]  # each [256, 512]
    out = np.concatenate(parts, axis=0).reshape(BATCH, SEQ, OUT_DIM)
    return np.ascontiguousarray(out.astype(np.float32))


# revision 15
# speedup vs baseline: 1.1155x; 1.1155x over previous
"""CharCNN embedder (ELMo-style) Trainium2 Bass kernel, v4.

Data parallel over 8 cores, 256 tokens each:
  - Embedding lookup as one-hot matmul: ids pre-replicated across the 128
    partitions host-side (pure input layout), one-hot via is_equal (DVE),
    embT.T @ onehot over 3 row-chunks -> xT [16, 12800] in conv layout.
  - im2col by 6 shifted SBUF->SBUF DMA copies -> X7 [118, 12800+pad] with
    indicator rows carrying -1e30 in the conv weights for invalid
    (channel, position) pairs so max-over-time needs no masking.
  - All 7 convs as one packed [118, 2048] bf16 matmul per (group, chunk);
    max-over-time as DVE reduce_max straight from PSUM (5-deep PSUM
    rotation so the PE is not gated on the drain).
  - Highway weight-stationary and channel-major: out[128 ch, 256 tok]
    accumulated over 16 K-chunks; bias fused into Scalar Relu/Sigmoid;
    gate math as three bf16 DVE tensor_tensor ops.  No transposes.
"""

import os
import numpy as np
import ml_dtypes

import concourse.bass as bass
import concourse.mybir as mybir
import concourse.tile as tile
from concourse.bass_utils import run_bass_kernel_spmd

F32 = mybir.dt.float32
BF16 = mybir.dt.bfloat16
FP16 = mybir.dt.float16
NPBF16 = ml_dtypes.bfloat16

CNN_OPTIONS = [(1, 32), (2, 32), (3, 64), (4, 128), (5, 256), (6, 512), (7, 1024)]
EMB_DIM = 16
N_CHARS = 262
MAX_CHARS = 50
N_FILTERS = 2048
OUT_DIM = 512
BATCH, SEQ = 4, 512
NCORES = 8
T_LOC = BATCH * SEQ // NCORES          # 256 tokens per core
COLS = T_LOC * MAX_CHARS               # 12800
COLS_PAD = COLS + 16                   # 12816
KMAX = 7
KROWS = EMB_DIM * KMAX                 # 112
KTOT = KROWS + 6                       # 118 (6 indicator rows for pos 44..49)
NCH = 512                              # xT build chunk width
NXCH = COLS // NCH                     # 25
TOKG = 8                               # tokens per conv chunk
NTG = T_LOC // TOKG                    # 32 conv N-chunks
CHUNK_K = [1, 4, 5, 5, 6, 6, 6, 6, 7, 7, 7, 7, 7, 7, 7, 7]
CHUNK_NP = [50 if k == 1 else (MAX_CHARS - k + 1) for k in CHUNK_K]
KC = 16                                # 2048/128 contraction chunks
NQ = 32                                # highway out-chunks (4096/128)


def _split_multi_waits(nc):
    """This walrus build encodes at most ONE sync-wait per instruction.
    Hoist extra waits onto dedicated NoOps ahead of the instruction."""
    ctr = [0]
    for f in nc.m.functions:
        for b in f.blocks:
            il = b.instructions
            if not any(
                i.sync_info is not None and len(i.sync_info.on_wait) > 1 for i in il
            ):
                continue
            new = []
            for ins in il:
                si = ins.sync_info
                if si is not None and len(si.on_wait) > 1:
                    waits = list(si.on_wait)
                    for w in waits[:-1]:
                        ctr[0] += 1
                        nop = mybir.InstNoOp(name=f"wsplit-{ctr[0]}", ins=[], outs=[])
                        nop.engine = ins.engine
                        nop.sync_info = mybir.SyncInfo(on_wait=[w], on_update=[])
                        new.append(nop)
                    ins.sync_info = mybir.SyncInfo(
                        on_wait=[waits[-1]], on_update=list(si.on_update)
                    )
                new.append(ins)
            b.instructions = new


def _build_program():
    nc = bass.Bass(target_bir_lowering=False)

    idsr_d = nc.dram_tensor("idsr", [128, COLS_PAD], FP16, kind="ExternalInput")
    iota_d = nc.dram_tensor("iota3", [128, 4], F32, kind="ExternalInput")
    embt_d = nc.dram_tensor("embt", [384, EMB_DIM], FP16, kind="ExternalInput")
    convw_d = nc.dram_tensor("convw", [KTOT, N_FILTERS], BF16, kind="ExternalInput")
    indic_d = nc.dram_tensor("indic", [6, COLS_PAD], BF16, kind="ExternalInput")
    cbias_d = nc.dram_tensor("cbias", [128, 16], F32, kind="ExternalInput")
    hw0_d = nc.dram_tensor("hw0", [NQ, 128, KC * 128], BF16, kind="ExternalInput")
    hw1_d = nc.dram_tensor("hw1", [NQ, 128, KC * 128], BF16, kind="ExternalInput")
    hb0_d = nc.dram_tensor("hb0", [128, NQ], F32, kind="ExternalInput")
    hb1_d = nc.dram_tensor("hb1", [128, NQ], F32, kind="ExternalInput")
    pw_d = nc.dram_tensor("pw", [KC, 128, 512], BF16, kind="ExternalInput")
    pb_d = nc.dram_tensor("pb", [1, 512], BF16, kind="ExternalInput")
    out_d = nc.dram_tensor("outT", [T_LOC, OUT_DIM], F32, kind="ExternalOutput")

    with tile.TileContext(nc) as tc:
        with (
            tc.tile_pool(name="const", bufs=1) as cpool,
            tc.tile_pool(name="oh", bufs=4) as ohpool,
            tc.tile_pool(name="wslab", bufs=6) as wpool,
            tc.tile_pool(name="elem", bufs=4) as epool,
            tc.tile_pool(name="outp", bufs=2) as outpool,
            tc.tile_pool(name="ps_cv", bufs=5, space="PSUM") as ps_cv,
            tc.tile_pool(name="ps_hw", bufs=2, space="PSUM") as ps_hw,
        ):
            # ---- constants in ----
            idsr_s = cpool.tile([128, COLS_PAD], FP16, tag="idsr")
            nc.sync.dma_start(idsr_s[:], idsr_d[:])
            iota_s = cpool.tile([128, 4], F32, tag="iota")
            nc.sync.dma_start(iota_s[:], iota_d[:])
            embt_s = cpool.tile([128, 3 * EMB_DIM], FP16, tag="embt")
            for r in range(3):
                nc.sync.dma_start(
                    embt_s[:, 16 * r : 16 * r + 16], embt_d[128 * r : 128 * r + 128, :]
                )
            convw_s = cpool.tile([KTOT, N_FILTERS], BF16, tag="convw")
            nc.sync.dma_start(convw_s[:], convw_d[:])
            cbias_s = cpool.tile([128, 16], F32, tag="cbias")
            nc.sync.dma_start(cbias_s[:], cbias_d[:])
            hb0_s = cpool.tile([128, NQ], F32, tag="hb0")
            nc.sync.dma_start(hb0_s[:], hb0_d[:])
            hb1_s = cpool.tile([128, NQ], F32, tag="hb1")
            nc.sync.dma_start(hb1_s[:], hb1_d[:])
            pb_s = cpool.tile([1, 512], BF16, tag="pb")
            nc.sync.dma_start(pb_s[:], pb_d[:])
            onesb_s = cpool.tile([1, 128], BF16, tag="onesb")
            nc.gpsimd.memset(onesb_s[:], 1.0)

            # X7: rows 0-15 xT base, 16-111 shifted copies, 112-117 indicators
            X7 = cpool.tile([KTOT, COLS_PAD], BF16, tag="X7")
            nc.sync.dma_start(X7[112:118, :], indic_d[:])
            nc.gpsimd.memset(X7[0:16, COLS:COLS_PAD], 0.0)

            hts = [
                cpool.tile([128, T_LOC], BF16, tag=f"hT{m}", name=f"hT{m}")
                for m in range(KC)
            ]
            h1ts = [
                cpool.tile([128, T_LOC], BF16, tag=f"h1T{m}", name=f"h1T{m}")
                for m in range(KC)
            ]
            h2ts = [
                cpool.tile([128, T_LOC], BF16, tag=f"h2T{m}", name=f"h2T{m}")
                for m in range(KC)
            ]

            # ---- embedding xT build ----
            def bphase(lo, hi):
                for n in range(lo, hi):
                    c0 = n * NCH
                    ohs = []
                    for r in range(3):
                        oh = ohpool.tile([128, NCH], FP16, tag=f"oh{r}", name=f"oh{r}_{n}")
                        nc.vector.tensor_scalar(
                            out=oh[:], in0=idsr_s[:, c0 : c0 + NCH],
                            scalar1=iota_s[:, r : r + 1],
                            scalar2=None, op0=mybir.AluOpType.is_equal,
                        )
                        ohs.append(oh)
                    px = ps_hw.tile([16, NCH], F32, space="PSUM", tag="hwp", name=f"px{n}")
                    for r in range(3):
                        nc.tensor.matmul(
                            px[:], embt_s[:, 16 * r : 16 * r + 16], ohs[r][:],
                            start=(r == 0), stop=(r == 2),
                        )
                    nc.scalar.copy(X7[0:16, c0 : c0 + NCH], px[:])

            def shifts(cl, cu):
                for j in range(1, KMAX):
                    nc.sync.dma_start(
                        X7[16 * j : 16 * j + 16, cl:cu], X7[0:16, cl + j : cu + j]
                    )

            # ---- conv + DVE max-over-time drain ----
            def conv_range(nlo, nhi):
                for nn in range(nlo, nhi):
                    c0 = nn * TOKG * MAX_CHARS
                    for m in range(16):
                        npos = CHUNK_NP[m]
                        fd = TOKG * npos
                        ps = ps_cv.tile(
                            [128, 512], F32, space="PSUM", tag="cv",
                            name=f"cv{m}_{nn}",
                        )
                        rhs = (
                            X7[0:KTOT, c0 : c0 + TOKG * MAX_CHARS]
                            .rearrange("p (t c) -> p t c", c=MAX_CHARS)[:, :, 0:npos]
                        )
                        nc.tensor.matmul(
                            ps[:, 0:fd], convw_s[:, 128 * m : 128 * m + 128], rhs,
                            start=True, stop=True,
                        )
                        nc.vector.reduce_max(
                            hts[m][:, TOKG * nn : TOKG * nn + TOKG],
                            ps[:, 0:fd].rearrange("p (t c) -> p t c", c=npos),
                            axis=mybir.AxisListType.X,
                        )

            def finish():
                for m in range(16):
                    nc.scalar.activation(
                        hts[m][:], hts[m][:], mybir.ActivationFunctionType.Relu,
                        bias=cbias_s[:, m : m + 1], scale=1.0,
                    )

            # ---- highway, weight-stationary / channel-major ----
            def hw_layer(lyr):
                src_ts = hts if lyr == 0 else h1ts
                dst_ts = h1ts if lyr == 0 else h2ts
                w_d = hw0_d if lyr == 0 else hw1_d
                hb_s = hb0_s if lyr == 0 else hb1_s
                for c in range(NQ // 2):
                    pshw = ps_hw.tile(
                        [128, 2 * T_LOC], F32, space="PSUM", tag="hwp",
                        name=f"hwp{lyr}_{c}",
                    )
                    for qq in range(2):
                        q = 2 * c + qq
                        slab = wpool.tile(
                            [128, KC * 128], BF16, tag="wslab", name=f"ws{lyr}_{q}"
                        )
                        nc.sync.dma_start(slab[:], w_d[q])
                        pq = pshw[:, qq * T_LOC : (qq + 1) * T_LOC]
                        for k in range(KC):
                            nc.tensor.matmul(
                                pq, slab[:, 128 * k : 128 * k + 128],
                                src_ts[k][:, 0:T_LOC],
                                start=(k == 0), stop=(k == KC - 1),
                            )
                    r_ = epool.tile([128, T_LOC], BF16, tag="relu", name=f"r{lyr}_{c}")
                    nc.scalar.activation(
                        r_[:], pshw[:, 0:T_LOC], mybir.ActivationFunctionType.Relu,
                        bias=hb_s[:, 2 * c : 2 * c + 1], scale=1.0,
                    )
                    g_ = epool.tile([128, T_LOC], BF16, tag="gate", name=f"g{lyr}_{c}")
                    nc.scalar.activation(
                        g_[:], pshw[:, T_LOC : 2 * T_LOC],
                        mybir.ActivationFunctionType.Sigmoid,
                        bias=hb_s[:, 2 * c + 1 : 2 * c + 2], scale=1.0,
                    )
                    t1 = epool.tile([128, T_LOC], BF16, tag="t1", name=f"c1_{lyr}_{c}")
                    nc.vector.tensor_tensor(
                        out=t1[:], in0=src_ts[c][:, 0:T_LOC], in1=r_[:],
                        op=mybir.AluOpType.subtract,
                    )
                    t2 = epool.tile([128, T_LOC], BF16, tag="t2", name=f"c2_{lyr}_{c}")
                    nc.vector.tensor_tensor(
                        out=t2[:], in0=g_[:], in1=t1[:], op=mybir.AluOpType.mult
                    )
                    nc.vector.tensor_tensor(
                        out=dst_ts[c][:, 0:T_LOC], in0=t2[:], in1=r_[:],
                        op=mybir.AluOpType.add,
                    )

            # ---- schedule ----
            bphase(0, 13)
            shifts(0, 6400)
            bphase(13, NXCH)
            shifts(6400, COLS)
            conv_range(0, NTG)
            finish()
            hw_layer(0)
            hw_layer(1)

            # ---- projection (token-major out) ----
            pslabs = []
            for k in range(KC):
                slab = wpool.tile([128, 512], BF16, tag="wslab", name=f"pws{k}")
                nc.sync.dma_start(slab[:], pw_d[k])
                pslabs.append(slab)
            for mt in range(2):
                ps = ps_hw.tile([128, 512], F32, space="PSUM", tag="hwp", name=f"pj{mt}")
                for k in range(KC):
                    nc.tensor.matmul(
                        ps[:], h2ts[k][:, 128 * mt : 128 * mt + 128], pslabs[k][:],
                        start=(k == 0), stop=False,
                    )
                nc.tensor.matmul(
                    ps[:], onesb_s[0:1, :], pb_s[0:1, :], start=False, stop=True
                )
                oc = outpool.tile([128, 512], F32, tag="out", name=f"oc{mt}")
                nc.scalar.copy(oc[:], ps[:])
                nc.sync.dma_start(out_d[128 * mt : 128 * mt + 128, :], oc[:])

    _split_multi_waits(nc)
    return nc


def _prep_weights(inputs):
    conv_ws = [np.asarray(inputs[f"conv_w{i}"], np.float32) for i in range(7)]
    conv_bs = [np.asarray(inputs[f"conv_b{i}"], np.float32) for i in range(7)]

    W7 = np.zeros((KTOT, N_FILTERS), np.float32)
    o0 = 0
    for (ksz, oc), w in zip(CNN_OPTIONS, conv_ws):
        for j in range(ksz):
            W7[16 * j : 16 * j + 16, o0 : o0 + oc] = w[:, :, j].T
        for i in range(6):
            if (44 + i) > (MAX_CHARS - ksz):
                W7[KROWS + i, o0 : o0 + oc] = -1e30
        o0 += oc

    b_all = np.concatenate(conv_bs)
    cbias = b_all.reshape(16, 128).T.astype(np.float32)

    indic = np.zeros((6, COLS_PAD), np.float32)
    for i in range(6):
        indic[i, (44 + i) : COLS : MAX_CHARS] = 1.0

    emb = np.asarray(inputs["emb"], np.float32)
    embt = np.zeros((384, EMB_DIM), np.float32)
    embt[:N_CHARS] = emb

    iota = np.zeros((128, 4), np.float32)
    for r in range(3):
        iota[:, r] = np.arange(128) + 128 * r
    iota[:, 3] = 1000.0  # never matches

    # weight-stationary highway slabs: out-chunk q=2c -> nonlin rows of block c,
    # q=2c+1 -> gate rows; slab[q][i, 128k+o] = W[rows_q[o], 128k+i]
    def hw_slabs(w, bvec):
        W = np.asarray(w, np.float32)           # [4096, 2048]
        bv = np.asarray(bvec, np.float32)
        slabs = np.zeros((NQ, 128, KC * 128), np.float32)
        hb = np.zeros((128, NQ), np.float32)
        for q in range(NQ):
            c = q // 2
            base = 128 * c if q % 2 == 0 else N_FILTERS + 128 * c
            rows = np.arange(base, base + 128)
            Wq = W[rows]
            slabs[q] = Wq.T.reshape(KC, 128, 128).transpose(1, 0, 2).reshape(128, KC * 128)
            hb[:, q] = bv[rows]
        return slabs.astype(NPBF16), hb

    hw0, hb0 = hw_slabs(inputs["hw_w0"], inputs["hw_b0"])
    hw1, hb1 = hw_slabs(inputs["hw_w1"], inputs["hw_b1"])
    pwt = np.asarray(inputs["proj_w"], np.float32).T
    pw = np.ascontiguousarray(pwt.reshape(KC, 128, 512)).astype(NPBF16)
    pb = np.asarray(inputs["proj_b"], np.float32)[None, :].astype(NPBF16)

    return {
        "iota3": iota,
        "embt": embt.astype(np.float16),
        "convw": W7.astype(NPBF16),
        "indic": indic.astype(NPBF16),
        "cbias": cbias,
        "hw0": hw0,
        "hw1": hw1,
        "hb0": hb0,
        "hb1": hb1,
        "pw": pw,
        "pb": pb,
    }


_NC_CACHE = []
LAST_RESULT = {}


def kernel(**inputs) -> np.ndarray:
    if not _NC_CACHE:
        _NC_CACHE.append(_build_program())
    nc = _NC_CACHE[0]

    shared = _prep_weights(inputs)
    ids = np.asarray(inputs["batch_ids"]).astype(np.int64).reshape(-1, MAX_CHARS)
    in_maps = []
    for core in range(NCORES):
        flat = ids[core * T_LOC : (core + 1) * T_LOC].reshape(-1)
        idsp = np.zeros((1, COLS_PAD), np.float16)
        idsp[0, :COLS] = flat.astype(np.float16)
        idsr = np.broadcast_to(idsp, (128, COLS_PAD)).copy()
        in_maps.append({"idsr": idsr, **shared})

    trace = bool(int(os.environ.get("KERNEL_TRACE", "0")))
    res = run_bass_kernel_spmd(
        nc, in_maps, core_ids=list(range(NCORES)), trace=trace
    )
    LAST_RESULT["exec_time_ns"] = res.exec_time_ns
    LAST_RESULT["trace"] = res.instructions_and_trace

    parts = [res.results[c]["outT"] for c in range(NCORES)]
    out = np.concatenate(parts, axis=0).reshape(BATCH, SEQ, OUT_DIM)
    return np.ascontiguousarray(out.astype(np.float32))
